# revision 31
# baseline (speedup 1.0000x reference)
"""Trainium2 Bass kernel: reversible block with 4-group dilated attention.

Reference computation (per batch b, seq n=8192, d_model=1024, DM=512):
    x1, x2 = split(x, 2, -1)
    y1 = x1 + AttnBlock(LN(x2; ln1)) ; y2 = x2 + FFN(LN(y1; ln2))
    out = concat(y1, y2)
AttnBlock(z) = LN(DilatedAttn(zWq+bq, zWk+bk, zWv+bv); ln_attn) @ Wo + bo
DilatedAttn: heads split into 4 groups of 2; group i attends within segments
of length s_i in (1024,2048,4096,8192) at stride r_i in (1,2,4,8), offset i%r_i.

Sharding: 8 cores = batch(2) x seq-quarter(4).  Each quarter of 2048 tokens is
self-contained for groups 0/1; for groups 2/3 the (strided) key/value source
rows are replicated to each core (host-side slicing), so there is no
cross-core communication.  Feature-major ([channel, token]) on device.

Perf notes vs the bf16 baseline:
  - fp8e4(DoubleRow) matmuls for: LN1 sum stats (host ships a pair-interleaved
    fp8 copy of x2/side rows), the attention AV+denominator matmul (exp tiles
    and V tiles in fp8), and the two FFN projections W1/W2 (fp8 weights, LN2
    output written fp8).  DR halves PE cycles and doubles contraction per
    matmul.
  - y2's residual + bias are folded to the host: the device ships only
    ffn(ln2(y1)) in bf16; host adds x2 + b2 in fp32 (also improves accuracy).
  - inputs are pre-cast to bf16 on the host so activation DMAs are plain
    HWDGE transfers on the SP queue; weight/const DMAs issue from the
    (otherwise idle) Pool SWDGE queue ahead of any partition_broadcasts.
  - NOTE: GPSIMD cannot access PSUM (BIR verifier), so evictions stay on
    ACT/DVE, balanced against the per-window bottleneck engine.
"""

import numpy as np
import ml_dtypes

import concourse.bass as bass
import concourse.mybir as mybir
import concourse.tile as tile
from concourse import bacc
from concourse.bass_utils import run_bass_kernel_spmd

F32 = mybir.dt.float32
BF16 = mybir.dt.bfloat16
F8 = mybir.dt.float8e4
DR = mybir.MatmulPerfMode.DoubleRow
AF = mybir.ActivationFunctionType
OP = mybir.AluOpType

NPF8 = ml_dtypes.float8_e4m3fn
NPBF = ml_dtypes.bfloat16

DM = 512              # working width seen by attention/FFN
HD = 64               # head dim
T_LOC = 2048          # tokens per core
T_G2 = 1024           # group-2 replicated key rows (one 4096-segment, stride 4)
T_G3 = 1024           # group-3 replicated key rows (whole seq, stride 8)
NCH = DM // 128       # 4 channel chunks
EPS = 1e-5
SCALE = 1.0 / 8.0     # 1/sqrt(HD)

# order of [512]-vectors packed into the "vecs"/"vecsT" inputs
VEC_NAMES = ["bq", "bk", "bv", "bo", "b1", "b2",
             "ln1_g", "ln1_b", "lna_g", "lna_b", "ln2_g", "ln2_b",
             "wkbar", "wvbar", "w1bar"]
NVEC = 16  # padded columns

WEIGHT_NAMES = ["Wq", "Wk", "Wv", "Wo"]


def ts(i, size):
    return slice(i * size, (i + 1) * size)


def _emit(ctx, tc, P):
    """Emit the whole per-core program into TileContext tc."""
    nc = tc.nc

    # ---------------- pools ----------------
    pool = lambda **kw: ctx.enter_context(tc.tile_pool(**kw))
    consts = pool(name="consts", bufs=1)     # vectors, ones, bvb
    wtagp = pool(name="wtagp", bufs=12)      # bf16 weight chunks (shared slots)
    wdrp = pool(name="wdrp", bufs=2)         # fp8 DR weights (W1, W2)
    stage = pool(name="stage", bufs=2)       # bf16 residual staging
    xbp = pool(name="xbp", bufs=4)           # bf16 x2-side inputs
    xdrp = pool(name="xdrp", bufs=1)         # fp8 DR copies of x2/side rows
    x2np = pool(name="x2np", bufs=4)         # LN1 outputs
    acts = pool(name="acts", bufs=11)        # large bf16 activations
    vtp = pool(name="vtp", bufs=1)           # fp8 V tiles per group
    otp = pool(name="otp", bufs=1)           # attention output (feature-major)
    sqp = pool(name="sqp", bufs=3)           # squared tiles for stats
    rmrp = pool(name="rmrp", bufs=2)         # R / MR stat rows (bf16, replicated)
    smp = pool(name="smp", bufs=2)           # small stat tiles
    ebp = pool(name="ebp", bufs=6)           # fp8 exp(S) tiles
    workp = pool(name="workp", bufs=2)       # LN apply temps (shared tag)
    ffn1p = pool(name="ffn1p", bufs=2)       # per-quarter LN2 outputs (fp8 DR)
    ffn2p = pool(name="ffn2p", bufs=2)       # per-quarter ReLU outputs (fp8 DR)
    y2bp = pool(name="y2bp", bufs=2)         # y2 partial staging (bf16)

    pmm = pool(name="pmm", bufs=4, space="PSUM")    # 4 banks: matmul outs + stats
    psb = pool(name="psb", bufs=2, space="PSUM")    # 4 banks: attention logits

    # ---------------- constants ----------------
    ones = consts.tile([128, 128], BF16, name="ones")
    nc.vector.memset(ones, 1.0)
    ones8 = consts.tile([128, 2, 128], F8, name="ones8")
    nc.vector.memset(ones8, 1.0)
    eps_col = consts.tile([128, 1], F32, name="eps_col")
    nc.vector.memset(eps_col, EPS)
    from concourse.masks import make_identity
    ident = consts.tile([128, 128], BF16, name="ident")
    make_identity(nc, ident[:])

    # weights -> bf16 tiles wb[name][k] : [128, DM] (plain bf16 DMA; the
    # params are already bf16 on the host).
    wb = {}

    def load_weight(w):
        if w in wb:
            return
        wb[w] = []
        for k in range(NCH):
            t = wtagp.tile([128, DM], BF16, tag="wtag", name=f"wb_{w}_{k}")
            nc.gpsimd.dma_start(out=t, in_=P[w][ts(k, 128), :])
            wb[w].append(t)

    wdr = {}

    def load_wdr(w):
        if w in wdr:
            return
        t = wdrp.tile([128, 2, 2, DM], F8, tag="wdr", name=f"wdr_{w}")
        nc.sync.dma_start(out=t, in_=P[w + "dr"][:, :, :, :])
        wdr[w] = t

    # per-channel vectors, channel-major: vst[k] = [128, NVEC]
    vst = []
    for k in range(NCH):
        t = consts.tile([128, NVEC], F32, name=f"vst_{k}")
        nc.gpsimd.dma_start(out=t, in_=P["vecsT"][ts(k, 128), :])
        vst.append(t)

    def vcol(name, k):
        return vst[k][:, VEC_NAMES.index(name):VEC_NAMES.index(name) + 1]

    # bf16 rows of the vec table (rank-1 lhsT operands; base partition 0)
    _vrow_tiles = {}

    def vrow(name):
        if name not in _vrow_tiles:
            i = VEC_NAMES.index(name)
            t = consts.tile([1, DM], BF16, name=f"vrow_{name}")
            nc.sync.dma_start(out=t, in_=P["rowsb"][i:i + 1, :])
            _vrow_tiles[name] = t
        return _vrow_tiles[name]

    # bv broadcast across partitions (needed in token-major V tiles):
    bvb = []
    bv_i = VEC_NAMES.index("bv")
    for k in range(NCH):
        t = consts.tile([128, 128], F32, name=f"bvb_{k}")
        nc.gpsimd.dma_start(out=t, in_=P["vecs"][bv_i:bv_i + 1, ts(k, 128)].to_broadcast([128, 128]))
        bvb.append(t)

    # attention output assembly tiles, bf16 feature-major [512, T_LOC];
    oT = [otp.tile([128, T_LOC], BF16, name=f"oT_{i}", tag=f"oT_{i}")
          for i in range(4)]

    # ---------------- LN helpers (feature-major) ----------------
    # LN output = (x - m) * R.  gamma pre-folded into projection weights,
    # beta into projection biases (host-side).
    def ln_alloc(name):
        R = rmrp.tile([128, T_LOC], BF16, tag="R", name=f"R_{name}")
        M = rmrp.tile([128, T_LOC], BF16, tag="M", name=f"M_{name}")
        return R, M

    def ln_stats_chunk(xb_tiles, tq, R, M, name, sq_act=False, x8=None):
        """One 512-column chunk of feature-major LN stats.
        x8: optional fp8 DR-paired copy [128, 2, 2, T] -> sum via 2 DR matmuls."""
        s_ps = pmm.tile([128, 512], F32, tag="mm", name=f"sps_{name}_{tq}")
        q_ps = pmm.tile([128, 512], F32, tag="mm", name=f"qps_{name}_{tq}")
        if x8 is not None:
            for j in range(2):
                nc.tensor.matmul(s_ps, lhsT=ones8, rhs=x8[:, :, j, ts(tq, 512)],
                                 start=(j == 0), stop=(j == 1), perf_mode=DR)
        for k in range(NCH):
            xsl = xb_tiles[k][:, ts(tq, 512)]
            sq = sqp.tile([128, 512], BF16, tag="sq", name=f"sq_{name}_{tq}_{k}")
            if sq_act:
                nc.scalar.activation(out=sq, in_=xsl, func=AF.Square)
            else:
                nc.vector.tensor_mul(out=sq, in0=xsl, in1=xsl)
            if x8 is None:
                nc.tensor.matmul(s_ps, lhsT=ones, rhs=xsl, start=(k == 0), stop=(k == NCH - 1))
            nc.tensor.matmul(q_ps, lhsT=ones, rhs=sq, start=(k == 0), stop=(k == NCH - 1))
        m_sl = M[:, ts(tq, 512)]
        nc.vector.tensor_scalar(out=m_sl, in0=s_ps, scalar1=1.0 / DM, scalar2=0.0,
                                op0=OP.mult, op1=OP.add)
        msq = smp.tile([128, 512], BF16, tag="msq", name=f"msq_{name}_{tq}")
        nc.vector.tensor_mul(out=msq, in0=m_sl, in1=m_sl)
        var = smp.tile([128, 512], F32, tag="var", name=f"var_{name}_{tq}")
        nc.vector.scalar_tensor_tensor(out=var, in0=q_ps, scalar=1.0 / DM, in1=msq,
                                       op0=OP.mult, op1=OP.subtract)
        lnv = smp.tile([128, 512], F32, tag="lnv", name=f"lnv_{name}_{tq}")
        nc.scalar.activation(out=lnv, in_=var, func=AF.Ln, bias=eps_col)
        nc.scalar.activation(out=R[:, ts(tq, 512)], in_=lnv, func=AF.Exp, scale=-0.5)

    def ln_stats(xb_tiles, T, name, sq_act=False, x8=None):
        R, M = ln_alloc(name)
        for tq in range(T // 512):
            ln_stats_chunk(xb_tiles, tq, R, M, name, sq_act=sq_act, x8=x8)
        return R, M

    def ln_apply_chunk(xb_tiles, R, M, outs, name, lo, width, mul_only=False):
        """out = (x - m) * R over columns [lo, lo+width), all NCH chunks."""
        c = slice(lo, lo + width)
        for k in range(NCH):
            if mul_only:
                nc.vector.tensor_mul(out=outs[k][:, c], in0=xb_tiles[k][:, c],
                                     in1=R[:, c])
                continue
            v = workp.tile([128, 1024], BF16, tag="uw", name=f"v_{name}_{k}_{lo}")
            nc.vector.tensor_sub(out=v[:, 0:width], in0=xb_tiles[k][:, c], in1=M[:, c])
            nc.vector.tensor_mul(out=outs[k][:, c], in0=v[:, 0:width], in1=R[:, c])

    def ln_apply(xb_tiles, R, M, out_pool, T, name, out_tag=None, mul_only=False):
        outs = [out_pool.tile([128, T], BF16, tag=(out_tag or "x2n"),
                              name=f"{name}_n_{k}") for k in range(NCH)]
        for lo in range(0, T, 1024):
            ln_apply_chunk(xb_tiles, R, M, outs, name, lo, min(1024, T - lo),
                           mul_only=mul_only)
        return outs

    def load_bf16(param, T, name):
        xb = []
        for k in range(NCH):
            t = xbp.tile([128, T], BF16, tag="xb", name=f"xb_{name}_{k}")
            nc.sync.dma_start(out=t, in_=param[ts(k, 128), :])
            xb.append(t)
        return xb

    def proj(dst, wname, bname, src_tiles, m, T, src_ap_fn=None, nq_range=None,
             rank1=None):
        """dst [128, T] bf16 = (src @ W[:, m-block] + b) feature-major.
        PSUM eviction + bias add on the Pool engine."""
        w = min(512, T)
        for nq in (range(T // w) if nq_range is None else nq_range):
            ps = pmm.tile([128, 512], F32, tag="mm", name=f"ps_{wname}_{m}_{nq}")
            for k in range(NCH):
                rhs = (src_ap_fn(src_tiles[k], nq) if src_ap_fn
                       else src_tiles[k][:, ts(nq, w)])
                nc.tensor.matmul(ps[:, 0:w], lhsT=wb[wname][k][:, ts(m, 128)], rhs=rhs,
                                 start=(k == 0), stop=(rank1 is None and k == NCH - 1))
            if rank1 is not None:
                row, mr = rank1
                nc.tensor.matmul(ps[:, 0:w], lhsT=row[0:1, ts(m, 128)],
                                 rhs=mr[0:1, ts(nq, w)], start=False, stop=True)
            nc.scalar.activation(out=dst[:, ts(nq, w)], in_=ps[:, 0:w],
                                 func=AF.Identity, bias=vcol(bname, m))

    # token-major fp8 V tiles with DoubleRow chunk pairing:
    # vt8[p, cpair, parity, h, 0:65]; the 65th col = 1.0 for the denominator.
    def mk_vt(n_chunks, group):
        # inner block padded 65->72 so the DR pair stride (2*72=144 bytes)
        # satisfies the LDWEIGHTS dual-fp8 step%16==0 ISA restriction
        vt = vtp.tile([128, n_chunks // 2, 2, 2, 72], F8, tag=f"vt{group}",
                      name=f"vt_{group}")
        nc.vector.memset(vt[:, :, :, :, 64:65], 1.0)
        return vt

    def fill_vt(vt, src_tiles, tok_ap_fn, group, chunks):
        for c in chunks:
            ps = pmm.tile([128, 128], F32, tag="mm", name=f"psv_{group}_{c}")
            for k in range(NCH):
                nc.tensor.matmul(ps, lhsT=tok_ap_fn(src_tiles[k], c),
                                 rhs=wb["Wv"][k][:, ts(group, 128)],
                                 start=(k == 0), stop=(k == NCH - 1))
            nc.vector.scalar_tensor_tensor(
                out=vt[:, c // 2, c % 2, :, 0:64],
                in0=ps[:].rearrange("p (h d) -> p h d", h=2),
                scalar=1.0,
                in1=bvb[group][:].rearrange("p (h d) -> p h d", h=2),
                op0=OP.bypass, op1=OP.add)

    def make_vt(src_tiles, tok_ap_fn, n_chunks, group):
        vt = mk_vt(n_chunks, group)
        fill_vt(vt, src_tiles, tok_ap_fn, group, range(n_chunks))
        return vt

    # ---------------- attention inner ----------------
    def do_attn(name, h, q_ap, o_ap, key_ap_fn, vtpair_fn, nq):
        kc_per_big = 1024 // nq
        ebs = []
        for big in range(8 // kc_per_big):
            sb = psb.tile([128, 1024], F32, tag="sbig", name=f"sb_{name}_{big}")
            for j in range(kc_per_big):
                kc = big * kc_per_big + j
                nc.tensor.matmul(sb[:, ts(j, nq)], lhsT=key_ap_fn(kc, h), rhs=q_ap,
                                 start=True, stop=True)
            eb = ebp.tile([128, 1024], F8, tag="eb", name=f"eb_{name}_{big}")
            nc.scalar.activation(out=eb, in_=sb, func=AF.Exp, scale=SCALE)
            ebs.append(eb)
        ops = pmm.tile([128, 512], F32, tag="mm", name=f"ops_{name}")
        for cp in range(4):
            big, m2 = divmod(2 * cp, kc_per_big)
            rhs = ebs[big][:, :].rearrange("p (j q) -> p j q", q=nq)[:, m2:m2 + 2, :]
            nc.tensor.matmul(ops[0:65, 0:nq], lhsT=vtpair_fn(cp),
                             rhs=rhs, start=(cp == 0), stop=(cp == 3),
                             perf_mode=DR)
        rc = smp.tile([1, 512], BF16, tag="rc", name=f"rc_{name}")
        with nc.allow_low_precision(reason="softmax denom reciprocal in bf16"):
            nc.vector.reciprocal(out=rc[:, 0:nq], in_=ops[64:65, 0:nq])
        rb = smp.tile([64, 512], BF16, tag="rb", name=f"rb_{name}")
        nc.gpsimd.partition_broadcast(rb[:, 0:nq], rc[0:1, 0:nq], 64)
        nc.vector.tensor_mul(out=o_ap, in0=ops[0:64, 0:nq], in1=rb[:, 0:nq])

    hs = lambda h: slice(h * 64, (h + 1) * 64)

    def dil_ap(tile_, r, off):
        return tile_[:, :].rearrange("p (t r) -> p t r", r=r)[:, :, off]

    # -------- phase 1: LN1 half 0 -> seg-0 Q/K/V -> seg-0 attention --------
    xb_loc = load_bf16(P["x2T"], T_LOC, "loc")
    x8_loc = xdrp.tile([128, 2, 2, T_LOC], F8, tag="x8loc", name="x8_loc")
    nc.sync.dma_start(out=x8_loc, in_=P["x2dr"][:, :, :, :])
    R_loc, M_loc = ln_alloc("loc")
    ln_stats_chunk(xb_loc, 0, R_loc, M_loc, "loc", sq_act=False, x8=x8_loc)
    ln_stats_chunk(xb_loc, 1, R_loc, M_loc, "loc", sq_act=False, x8=x8_loc)
    x2n = [x2np.tile([128, T_LOC], BF16, tag="x2n", name=f"loc_n_{k}")
           for k in range(NCH)]
    ln_apply_chunk(xb_loc, R_loc, M_loc, x2n, "loc", 0, 1024)

    q_sizes = [T_LOC, 1024, 512, 256]
    QT = [acts.tile([128, q_sizes[g]], BF16, tag="act5", name=f"QT_{g}")
          for g in range(4)]
    load_weight("Wq")
    proj(QT[0], "Wq", "bq", x2n, 0, T_LOC, nq_range=(0, 1))
    load_weight("Wk")
    KT0 = acts.tile([128, T_LOC], BF16, tag="act5", name="KT_0")
    proj(KT0, "Wk", "bk", x2n, 0, T_LOC, nq_range=(0, 1))
    load_weight("Wv")
    vt0 = mk_vt(16, 0)
    fill_vt(vt0, x2n, lambda t, c: t[:, ts(c, 128)], 0, range(8))
    ln_stats_chunk(xb_loc, 2, R_loc, M_loc, "loc", sq_act=False, x8=x8_loc)
    ln_stats_chunk(xb_loc, 3, R_loc, M_loc, "loc", sq_act=False, x8=x8_loc)

    def g0_attn(seg, h, qc):
        do_attn(f"g0_{seg}_{h}_{qc}", h,
                QT[0][hs(h), seg * 1024 + qc * 512: seg * 1024 + (qc + 1) * 512],
                oT[0][hs(h), seg * 1024 + qc * 512: seg * 1024 + (qc + 1) * 512],
                lambda kc, h_: KT0[hs(h_), seg * 1024 + kc * 128: seg * 1024 + (kc + 1) * 128],
                lambda cp: vt0[:, seg * 4 + cp, :, h, 0:65], 512)

    # side x loads + gap memsets issue early
    xb_g2 = load_bf16(P["xg2T"], 1024, "g2")
    xb_g3 = load_bf16(P["xg3T"], 1024, "g3")
    x8_g2 = xdrp.tile([128, 2, 2, T_G2], F8, tag="x8g2", name="x8_g2")
    nc.sync.dma_start(out=x8_g2, in_=P["xg2dr"][:, :, :, :])
    x8_g3 = xdrp.tile([128, 2, 2, T_G3], F8, tag="x8g3", name="x8_g3")
    nc.sync.dma_start(out=x8_g3, in_=P["xg3dr"][:, :, :, :])
    for i in range(1, 4):
        nc.gpsimd.memset(oT[i], 0.0)

    ln_apply_chunk(xb_loc, R_loc, M_loc, x2n, "loc", 1024, 1024)

    for qc in range(2):
        for h in range(2):
            g0_attn(0, h, qc)

    # -------- phase 2: remaining local Q/K/V --------
    proj(QT[0], "Wq", "bq", x2n, 0, T_LOC, nq_range=(2, 3))
    proj(QT[1], "Wq", "bq", x2n, 1, 1024,
         lambda t, nq: dil_ap(t, 2, 1)[:, ts(nq, 512)])
    proj(QT[2], "Wq", "bq", x2n, 2, 512,
         lambda t, nq: dil_ap(t, 4, 2)[:, ts(nq, 512)])
    proj(QT[3], "Wq", "bq", x2n, 3, 256,
         lambda t, nq: dil_ap(t, 8, 3)[:, 0:256])
    proj(KT0, "Wk", "bk", x2n, 0, T_LOC, nq_range=(2, 3))
    KTg1 = acts.tile([128, 1024], BF16, tag="act5", name="KT_1")
    proj(KTg1, "Wk", "bk", x2n, 1, 1024,
         lambda t, nq: dil_ap(t, 2, 1)[:, ts(nq, 512)])
    fill_vt(vt0, x2n, lambda t, c: t[:, ts(c, 128)], 0, range(8, 16))
    vt1 = make_vt(x2n, lambda t, c: dil_ap(t, 2, 1)[:, ts(c, 128)], 8, 1)

    # -------- phase 3: side groups 2/3 (chain + attention) --------
    def side_group(xbg, x8g, group, name):
        Rg, Mg = ln_stats(xbg, 1024, name, sq_act=True, x8=x8g)
        x2ng = ln_apply(xbg, Rg, Mg, x2np, 1024, name,
                        out_tag="x2n_side")
        ktg = acts.tile([128, 1024], BF16, tag="act5", name=f"KTg_{group}")
        proj(ktg, "Wk", "bk", x2ng, group, 1024)
        vtg = make_vt(x2ng, lambda t, c: t[:, ts(c, 128)], 8, group)
        return ktg, vtg

    KTg2, vt2 = side_group(xb_g2, x8_g2, 2, "g2")
    for h in range(2):
        do_attn(f"g2_{h}", h, QT[2][hs(h), 0:512],
                dil_ap(oT[2], 4, 2)[hs(h), 0:512],
                lambda kc, h_: KTg2[hs(h_), ts(kc, 128)],
                lambda cp: vt2[:, cp, :, h, 0:65], 512)
    KTg3, vt3 = side_group(xb_g3, x8_g3, 3, "g3")

    # ------------- phase 4: local attention + LN_attn stats (interleaved) ----
    load_weight("Wo")
    R_a, M_a = ln_alloc("lna")

    def g1_attn(h, qc):
        do_attn(f"g1_{h}_{qc}", h, QT[1][hs(h), ts(qc, 512)],
                dil_ap(oT[1], 2, 1)[hs(h), ts(qc, 512)],
                lambda kc, h_: KTg1[hs(h_), ts(kc, 128)],
                lambda cp: vt1[:, cp, :, h, 0:65], 512)

    o_n = [acts.tile([128, T_LOC], BF16, tag="act5", name=f"lna_n_{k}")
           for k in range(NCH)]
    y1b = [acts.tile([128, T_LOC], BF16, tag="act5", name=f"y1b_{m}")
           for m in range(4)]
    load_wdr("W1")
    R_2, M_2 = ln_alloc("ln2")
    mr2 = rmrp.tile([1, T_LOC], BF16, tag="mr2", name="mr2")

    def phase5_half(half):
        ln_apply_chunk(oT, R_a, M_a, o_n, "lna", half * 1024, 1024)
        for nq in (2 * half, 2 * half + 1):
            for m in range(4):
                ps = pmm.tile([128, 512], F32, tag="mm", name=f"pso_{m}_{nq}")
                for k in range(NCH):
                    nc.tensor.matmul(ps, lhsT=wb["Wo"][k][:, ts(m, 128)],
                                     rhs=o_n[k][:, ts(nq, 512)],
                                     start=(k == 0), stop=False)
                x1f = stage.tile([128, 512], BF16, tag="x1stage", name=f"x1f_{m}_{nq}")
                nc.sync.dma_start(out=x1f, in_=P["x1T"][ts(m, 128), ts(nq, 512)])
                nc.tensor.matmul(ps, lhsT=ident, rhs=x1f, start=False, stop=True)
                nc.scalar.activation(out=y1b[m][:, ts(nq, 512)], in_=ps,
                                     func=AF.Identity, bias=vcol("bo", m))
                nc.sync.dma_start(out=P["out1"][ts(m, 128), ts(nq, 512)],
                                  in_=y1b[m][:, ts(nq, 512)])
            ln_stats_chunk(y1b, nq, R_2, M_2, "ln2")
            nc.vector.tensor_mul(out=mr2[0:1, ts(nq, 512)],
                                 in0=M_2[0:1, ts(nq, 512)],
                                 in1=R_2[0:1, ts(nq, 512)])

    # ---------------- phase 6: LN2 + FFN + y2 partial, per-quarter ----------
    def phase6_quarter(nq):
        c = slice(nq * 512, nq * 512 + 512)
        y1n8 = ffn1p.tile([128, 2, 2, 512], F8, tag="y1n", name=f"ln2n_{nq}")
        for k in range(NCH):
            nc.vector.tensor_mul(out=y1n8[:, k % 2, k // 2, :],
                                 in0=y1b[k][:, c], in1=R_2[:, c])
        hb8 = ffn2p.tile([128, 2, 2, 512], F8, tag="hb", name=f"hb_{nq}")
        load_wdr("W2")
        for m in range(4):
            ps = pmm.tile([128, 512], F32, tag="mm", name=f"psh_{m}_{nq}")
            for j in range(2):
                nc.tensor.matmul(ps, lhsT=wdr["W1"][:, :, j, ts(m, 128)],
                                 rhs=y1n8[:, :, j, :],
                                 start=(j == 0), stop=False, perf_mode=DR)
            nc.tensor.matmul(ps, lhsT=vrow("w1bar")[:, ts(m, 128)],
                             rhs=mr2[0:1, c], start=False, stop=True)
            nc.scalar.activation(out=hb8[:, m % 2, m // 2, :], in_=ps, func=AF.Relu,
                                 bias=vcol("b1", m))
        for m in range(4):
            ps = pmm.tile([128, 512], F32, tag="mm", name=f"psy2_{m}_{nq}")
            for j in range(2):
                nc.tensor.matmul(ps, lhsT=wdr["W2"][:, :, j, ts(m, 128)],
                                 rhs=hb8[:, :, j, :],
                                 start=(j == 0), stop=(j == 1), perf_mode=DR)
            y2b = y2bp.tile([128, 512], BF16, tag="y2b", name=f"y2b_{m}_{nq}")
            nc.vector.tensor_copy(out=y2b, in_=ps)
            nc.sync.dma_start(out=P["out2"][ts(m, 128), ts(nq, 512)], in_=y2b)

    # columns 0:1024 of local attention, then their LN_attn stats + phase 5/6,
    # overlapped with the second half of attention.
    for h in range(2):
        g1_attn(h, 0)
    for h in range(2):
        do_attn(f"g3_{h}", h, QT[3][hs(h), 0:256],
                dil_ap(oT[3], 8, 3)[hs(h), 0:256],
                lambda kc, h_: KTg3[hs(h_), ts(kc, 128)],
                lambda cp: vt3[:, cp, :, h, 0:65], 256)
    ln_stats_chunk(oT, 0, R_a, M_a, "lna")
    ln_stats_chunk(oT, 1, R_a, M_a, "lna")
    for qc in range(2):
        for h in range(2):
            g0_attn(1, h, qc)
    for h in range(2):
        g1_attn(h, 1)
    ln_stats_chunk(oT, 2, R_a, M_a, "lna")
    ln_stats_chunk(oT, 3, R_a, M_a, "lna")
    phase5_half(0)
    phase6_quarter(0)
    phase6_quarter(1)
    phase5_half(1)
    phase6_quarter(2)
    phase6_quarter(3)


_ACT_PATCHED = False


def _patch_act_tables():
    """Make every activation func we use resolve to the one table set that
    contains them all (natural_log_exp_and_others), so the act-table-load
    pass emits a single load instead of thrashing Ln<->Exp (~1.3us each)."""
    global _ACT_PATCHED
    if _ACT_PATCHED:
        return
    _ACT_PATCHED = True
    import concourse.hw_specs as _hw
    _orig = _hw.get_activation_tables
    mine = {AF.Exp, AF.Ln, AF.Relu, AF.Identity, AF.Copy}

    def _patched(arch):
        t = _orig(arch)
        return {name: (s if name == "natural_log_exp_and_others" else s - mine)
                for name, s in t.items()}

    bacc.get_activation_tables = _patched


def build_nc():
    _patch_act_tables()
    nc = bacc.Bacc()
    P = {}
    P["x1T"] = nc.declare_dram_parameter("x1T", [DM, T_LOC], BF16, isOutput=False)
    P["x2T"] = nc.declare_dram_parameter("x2T", [DM, T_LOC], BF16, isOutput=False)
    P["xg2T"] = nc.declare_dram_parameter("xg2T", [DM, T_G2], BF16, isOutput=False)
    P["xg3T"] = nc.declare_dram_parameter("xg3T", [DM, T_G3], BF16, isOutput=False)
    P["x2dr"] = nc.declare_dram_parameter("x2dr", [128, 2, 2, T_LOC], F8, isOutput=False)
    P["xg2dr"] = nc.declare_dram_parameter("xg2dr", [128, 2, 2, T_G2], F8, isOutput=False)
    P["xg3dr"] = nc.declare_dram_parameter("xg3dr", [128, 2, 2, T_G3], F8, isOutput=False)
    for w in WEIGHT_NAMES:
        P[w] = nc.declare_dram_parameter(w, [DM, DM], BF16, isOutput=False)
    P["W1dr"] = nc.declare_dram_parameter("W1dr", [128, 2, 2, DM], F8, isOutput=False)
    P["W2dr"] = nc.declare_dram_parameter("W2dr", [128, 2, 2, DM], F8, isOutput=False)
    P["vecsT"] = nc.declare_dram_parameter("vecsT", [DM, NVEC], F32, isOutput=False)
    P["vecs"] = nc.declare_dram_parameter("vecs", [NVEC, DM], F32, isOutput=False)
    P["rowsb"] = nc.declare_dram_parameter("rowsb", [NVEC, DM], BF16, isOutput=False)
    P["out1"] = nc.declare_dram_parameter("out1", [DM, T_LOC], BF16, isOutput=True)
    P["out2"] = nc.declare_dram_parameter("out2", [DM, T_LOC], BF16, isOutput=True)

    from contextlib import ExitStack
    with tile.TileContext(nc) as tc:
        with ExitStack() as ctx:
            _emit(ctx, tc, P)
    nc.finalize()
    return nc


def _dr_pack(a):
    """[512, N...] fp32 -> fp8 [128, 2, 2, N] with DR pair interleave:
    out[p, i, j] = a[(2j+i)*128 + p]."""
    a4 = a.reshape(2, 2, 128, -1)  # [j, i, p, N]
    return np.ascontiguousarray(a4.transpose(2, 1, 0, 3)).astype(NPF8)


def make_in_maps(x, Wq, bq, Wk, bk, Wv, bv, ln_attn_g, ln_attn_b, Wo, bo,
                 W1, b1, W2, b2, ln1_g, ln1_b, ln2_g, ln2_b):
    """Shard the full inputs into 8 per-core input maps.

    LN gamma/beta are folded into the consumer projections here (pure host
    numpy): W' = g*W and b' = b + W^T ln_b, so the device LN apply is just
    (x - mean) * rstd."""
    x = np.ascontiguousarray(np.asarray(x, dtype=np.float32))
    f32 = lambda a: np.asarray(a, dtype=np.float32)
    Wq, Wk, Wv, Wo, W1, W2 = (f32(W) for W in (Wq, Wk, Wv, Wo, W1, W2))
    bq, bk, bv, bo, b1, b2 = (f32(b) for b in (bq, bk, bv, bo, b1, b2))
    b2_orig = b2.copy()
    g1, b1n = f32(ln1_g), f32(ln1_b)
    ga, ban = f32(ln_attn_g), f32(ln_attn_b)
    g2, b2n = f32(ln2_g), f32(ln2_b)
    cc = np.ascontiguousarray
    Ws = {"Wq": cc(g1[:, None] * Wq), "Wk": cc(g1[:, None] * Wk),
          "Wv": cc(g1[:, None] * Wv), "Wo": cc(ga[:, None] * Wo)}
    W1s = cc(g2[:, None] * W1)
    bq = bq + Wq.T @ b1n
    bk = bk + Wk.T @ b1n
    bv = bv + Wv.T @ b1n
    bo = bo + Wo.T @ ban
    b1 = b1 + W1.T @ b2n
    W1dr = _dr_pack(W1s)
    W2dr = _dr_pack(W2)
    # rank-1 mean corrections use the column sums of the scaled weights;
    # for the fp8 W1 use the quantized values so the correction is exact.
    wkbar = -Ws["Wk"].sum(axis=0)
    wvbar = -Ws["Wv"].sum(axis=0)
    w1bar = -W1dr.astype(np.float32).sum(axis=(0, 1, 2))
    vec_vals = [bq, bk, bv, bo, b1, b2, ln1_g, ln1_b, ln_attn_g, ln_attn_b,
                ln2_g, ln2_b, wkbar, wvbar, w1bar]
    vecs = np.zeros((NVEC, DM), np.float32)
    for i, v in enumerate(vec_vals):
        vecs[i] = np.asarray(v, dtype=np.float32)
    vecsT = np.ascontiguousarray(vecs.T)
    rowsb = vecs.astype(NPBF)
    Wsb = {k: v.astype(NPBF) for k, v in Ws.items()}

    bf = lambda a: np.ascontiguousarray(a).astype(NPBF)
    global _ASSEMBLE_CTX
    _ASSEMBLE_CTX = {"b2": b2_orig, "x2_parts": []}
    in_maps = []
    for core in range(8):
        b, qt = divmod(core, 4)
        t0 = qt * T_LOC
        x1 = x[b, t0:t0 + T_LOC, 0:DM]
        x2 = x[b, t0:t0 + T_LOC, DM:2 * DM]
        seg2 = qt // 2
        xg2 = x[b, seg2 * 4096 + 2: (seg2 + 1) * 4096: 4, DM:2 * DM]
        xg3 = x[b, 3::8, DM:2 * DM]
        x2T = np.ascontiguousarray(x2.T)
        xg2T = np.ascontiguousarray(xg2.T)
        xg3T = np.ascontiguousarray(xg3.T)
        _ASSEMBLE_CTX["x2_parts"].append(np.ascontiguousarray(x2))
        m = {
            "x1T": bf(x1.T),
            "x2T": x2T.astype(NPBF),
            "xg2T": xg2T.astype(NPBF),
            "xg3T": xg3T.astype(NPBF),
            "x2dr": _dr_pack(x2T),
            "xg2dr": _dr_pack(xg2T),
            "xg3dr": _dr_pack(xg3T),
            "W1dr": W1dr,
            "W2dr": W2dr,
            "vecsT": vecsT,
            "vecs": vecs,
            "rowsb": rowsb,
        }
        m.update(Wsb)
        in_maps.append(m)
    return in_maps


_NC_CACHE = None


def get_nc():
    global _NC_CACHE
    if _NC_CACHE is None:
        _NC_CACHE = build_nc()
    return _NC_CACHE


def _execute(in_maps, trace=False, **kwargs):
    nc = get_nc()
    return run_bass_kernel_spmd(nc, in_maps, core_ids=list(range(8)),
                                trace=trace, **kwargs)


_ASSEMBLE_CTX = None


def assemble_output(results):
    ctx = _ASSEMBLE_CTX
    out = np.empty((2, 8192, 2 * DM), np.float32)
    for core in range(8):
        b, qt = divmod(core, 4)
        sl = slice(qt * T_LOC, (qt + 1) * T_LOC)
        out[b, sl, 0:DM] = np.asarray(results[core]["out1"], dtype=np.float32).T
        y2p = np.asarray(results[core]["out2"], dtype=np.float32).T
        out[b, sl, DM:] = ctx["x2_parts"][core] + ctx["b2"][None, :] + y2p
    return out


def kernel(**inputs):
    in_maps = make_in_maps(**inputs)
    res = _execute(in_maps)
    return assemble_output(res.results)
